# revision 31
# baseline (speedup 1.0000x reference)
"""DiffAttn2d TRN2 Bass kernel.

Sharding: 8 cores = 2 (batch) x 4 (head-groups of 2 heads / 4 doubled-heads).
Per core, everything is computed channel-major (scores transposed: [key, query])
so softmax needs no transposes:
  - dots^T via row-packed K=16 fp32r matmuls (4 doubled heads in 4 PE row groups)
  - one ACT exp pass PSUM->SBUF (bf16), the only O(n^2) elementwise op
  - attn @ v as Z^T = [v | ones]^T @ exp^T (bf16, col-block packed 2 dheads/matmul);
    the ones column yields the softmax denominators for free
  - u = Z0/s0 - lam*Z1/s1 via reciprocal rows (exp(-ln(s))) broadcast with
    tiny selector matmuls, folded with a +/-1 matmul
  - LayerNorm stats via ones-matmuls; rsqrt via exp(-0.5*ln(var+eps))
  - sigmoid gating via exp/ln (same ACT table set as exp: zero table switches)
  - y^T = Wout^T @ gated, row-parallel partials summed on host (+ bout)
"""
import sys
sys.path.insert(0, "/opt/trn_rl_repo")

import math
import numpy as np
import ml_dtypes

import concourse.bass as bass
import concourse.bacc as bacc_mod
import concourse.mybir as mybir
from concourse.tile import TileContext
from concourse.bass_utils import run_bass_kernel_spmd

F = mybir.dt.float32
R = mybir.dt.float32r
BF = mybir.dt.bfloat16
AF = mybir.ActivationFunctionType
AL = mybir.AluOpType

H, DH = 8, 16
DEPTH = 1
LAMBDA_INIT = 0.8 - 0.6 * math.exp(-0.3 * DEPTH)
LN_EPS = 1e-5
B, N, DIM = 2, 2048, 256
NC = 8

_cached = {}


def build_kernel():
    nc = bacc_mod.Bacc()
    xT = nc.declare_dram_parameter("xT", [DIM, N], R, isOutput=False)
    wqp = nc.declare_dram_parameter("wqp", [DIM, 128], R, isOutput=False)
    wkp = nc.declare_dram_parameter("wkp", [DIM, 128], R, isOutput=False)
    wv = nc.declare_dram_parameter("wv", [DIM, 64], R, isOutput=False)
    wgc = nc.declare_dram_parameter("wgc", [DIM, 64], R, isOutput=False)
    wout = nc.declare_dram_parameter("wout", [32, 2, 256], R, isOutput=False)  # [ch, bank, out]
    sel = nc.declare_dram_parameter("sel", [4, 256], R, isOutput=False)        # [:, 0:128]=A, [:,128:]=B
    fold = nc.declare_dram_parameter("fold", [128, 32], R, isOutput=False)
    smu = nc.declare_dram_parameter("smu", [32, 4], R, isOutput=False)         # cols 0:2 mu-pass, 2:4 m2-pass
    ones32 = nc.declare_dram_parameter("ones32", [1, 32], R, isOutput=False)
    onescol = nc.declare_dram_parameter("onescol", [128, 32], BF, isOutput=False)
    gam = nc.declare_dram_parameter("gam", [32, 1], F, isOutput=False)
    bet = nc.declare_dram_parameter("bet", [32, 1], F, isOutput=False)
    nbg = nc.declare_dram_parameter("nbg", [32, 2], F, isOutput=False)
    epsc = nc.declare_dram_parameter("epsc", [1, 1], F, isOutput=False)
    yT = nc.declare_dram_parameter("yT", [DIM, N], F, isOutput=True)

    with TileContext(nc) as tc:
        with tc.tile_pool(name="pers", bufs=1) as pers, \
             tc.tile_pool(name="work", bufs=6) as work, \
             tc.tile_pool(name="epi", bufs=2) as epi, \
             tc.tile_pool(name="epi1", bufs=1) as epi1, \
             tc.tile_pool(name="zsp", bufs=4) as zsp, \
             tc.tile_pool(name="dp", bufs=2, space="PSUM") as dp, \
             tc.tile_pool(name="zp", bufs=2, space="PSUM") as zp:

            # ---------------- load persistent data ----------------
            xt = pers.tile([128, 2, N], R, tag="xt")      # [:, f, :] feature chunk f
            xTr = xT.rearrange("(f p) n -> p f n", p=128)
            for it in range(4):
                for f in range(2):
                    nc.sync.dma_start(out=xt[:, f, it * 512:(it + 1) * 512],
                                      in_=xTr[:, f, it * 512:(it + 1) * 512])
            twqp = pers.tile([128, 2, 128], R, tag="twqp")
            nc.sync.dma_start(out=twqp[:], in_=wqp.rearrange("(f p) m -> p f m", p=128))
            twkp = pers.tile([128, 2, 128], R, tag="twkp")
            nc.sync.dma_start(out=twkp[:], in_=wkp.rearrange("(f p) m -> p f m", p=128))
            twv = pers.tile([128, 2, 64], R, tag="twv")
            nc.sync.dma_start(out=twv[:], in_=wv.rearrange("(f p) m -> p f m", p=128))
            twg = pers.tile([128, 2, 64], R, tag="twg")
            nc.sync.dma_start(out=twg[:], in_=wgc.rearrange("(f p) m -> p f m", p=128))
            twout = pers.tile([32, 2, 256], R, tag="twout")
            nc.sync.dma_start(out=twout[:], in_=wout[:])
            tsel = pers.tile([4, 256], R, tag="tsel")
            nc.sync.dma_start(out=tsel[:], in_=sel[:])
            tfold = pers.tile([128, 32], R, tag="tfold")
            nc.sync.dma_start(out=tfold[:], in_=fold[:])
            tsmu = pers.tile([32, 4], R, tag="tsmu")
            nc.sync.dma_start(out=tsmu[:], in_=smu[:])
            tones32 = pers.tile([1, 32], R, tag="tones32")
            nc.sync.dma_start(out=tones32[:], in_=ones32[:])
            tgam = pers.tile([32, 1], F, tag="tgam")
            nc.sync.dma_start(out=tgam[:], in_=gam[:])
            tbet = pers.tile([32, 1], F, tag="tbet")
            nc.sync.dma_start(out=tbet[:], in_=bet[:])
            tnbg = pers.tile([32, 2], F, tag="tnbg")
            nc.sync.dma_start(out=tnbg[:], in_=nbg[:])
            teps = pers.tile([1, 1], F, tag="teps")
            nc.sync.dma_start(out=teps[:], in_=epsc[:])

            # ---------------- projections ----------------
            # q^T / k^T packed: partition 32d+j (j<16) = channel j of dhead d
            qTp = pers.tile([128, N], R, tag="qTp")
            kTp = pers.tile([128, N], R, tag="kTp")
            vpp = pers.tile([128, 16, 128], BF, tag="vpp")

            def proj_qk(dst, w, it):
                ps = dp.tile([128, 1024], F, tag="dots")
                for f in range(2):
                    nc.tensor.matmul(ps[:, 0:512], w[:, f, :], xt[:, f, it * 512:(it + 1) * 512],
                                     start=(f == 0), stop=(f == 1))
                nc.vector.tensor_copy(dst[:, it * 512:(it + 1) * 512], ps[:, 0:512])

            def proj_v(jc):
                ps = dp.tile([128, 1024], F, tag="dots")
                for f in range(2):
                    nc.tensor.matmul(ps[:, 0:64], xt[:, f, jc * 128:(jc + 1) * 128], twv[:, f, :],
                                     start=(f == 0), stop=(f == 1))
                nc.vector.tensor_copy(vpp[:, jc, 0:32], ps[:, 0:32])
                nc.vector.tensor_copy(vpp[:, jc, 64:96], ps[:, 32:64])
                nc.sync.dma_start(out=vpp[:, jc, 32:64], in_=onescol[:])
                nc.sync.dma_start(out=vpp[:, jc, 96:128], in_=onescol[:])

            proj_qk(kTp, twkp, 0)
            proj_qk(qTp, twqp, 0)
            proj_qk(qTp, twqp, 1)
            for jc in range(4):
                proj_v(jc)
            for it in range(1, 4):
                proj_qk(kTp, twkp, it)
            proj_qk(qTp, twqp, 2)
            proj_qk(qTp, twqp, 3)
            for jc in range(4, 16):
                proj_v(jc)

            # ---------------- main: attention ----------------
            saved = {}

            def attention(ip, interleave=None):
                i0 = ip * 1024
                zA = zp.tile([128, 1024], F, tag="z")
                zB = zp.tile([128, 1024], F, tag="z")
                zbank = (zA, zA, zB, zB)
                for jc in range(16):
                    if interleave is not None and jc >= 2:
                        next(interleave, None)
                    j0 = jc * 128
                    for pair in range(2):
                        ebfs = []
                        dts = []
                        for dd in range(2):
                            d = 2 * pair + dd
                            r0 = 32 * d
                            dt_ = dp.tile([128, 1024], F, tag="dots")
                            for h in range(2):
                                nc.tensor.matmul(
                                    dt_[:, h * 512:(h + 1) * 512],
                                    kTp[r0:r0 + 16, j0:j0 + 128],
                                    qTp[r0:r0 + 16, i0 + h * 512:i0 + (h + 1) * 512],
                                    start=True, stop=True,
                                    tile_position=(r0, 0),
                                )
                            eb = work.tile([128, 1024], BF, tag="ebf")
                            nc.scalar.activation(eb[:], dt_[:], AF.Exp)
                            ebfs.append(eb)
                            dts.append(dt_)
                        for dd in range(2):
                            d = 2 * pair + dd
                            zt = zbank[d]
                            for h in range(2):
                                nc.tensor.matmul(
                                    zt[64 * dd:64 * dd + 64, h * 512:(h + 1) * 512],
                                    vpp[:, jc, 64 * pair:64 * pair + 64],
                                    ebfs[dd][:, h * 512:(h + 1) * 512],
                                    start=(jc == 0), stop=(jc == 15),
                                    tile_position=(0, 64 * dd),
                                    skip_group_check=True,
                                )

                # hand Z off to SBUF immediately so the Z psum slots free up
                zsA = zsp.tile([128, 1024], F, tag="zs")
                zsB = zsp.tile([128, 1024], F, tag="zs")
                nc.vector.tensor_copy(zsA[:], zA[:])
                nc.vector.tensor_copy(zsB[:], zB[:])
                srows = epi.tile([4, 1024], F, tag="srows")
                nc.sync.dma_start(out=srows[0:2, :], in_=zsA[32:128:64, :])
                nc.sync.dma_start(out=srows[2:4, :], in_=zsB[32:128:64, :])
                saved[ip] = (zsA, zsB, srows)

            def epilogue(ip):
                i0 = ip * 1024
                zsA, zsB, srows = saved[ip]
                rinv = epi1.tile([4, 1024], R, tag="rinv")
                nc.scalar.activation(srows[:], srows[:], AF.Ln)
                nc.scalar.activation(rinv[:], srows[:], AF.Exp, scale=-1.0)
                yield

                us = epi1.tile([32, 2048], R, tag="us")
                for b, zs in ((0, zsA), (1, zsB)):
                    bf_ = dp.tile([128, 1024], F, tag="dots")
                    for nt in range(2):
                        nc.tensor.matmul(bf_[:, nt * 512:(nt + 1) * 512],
                                         tsel[:, 128 * b:128 * (b + 1)],
                                         rinv[:, nt * 512:(nt + 1) * 512],
                                         start=True, stop=True)
                    tt = epi1.tile([128, 1024], R, tag="tt")
                    nc.vector.tensor_tensor(tt[:], zs[:], bf_[:], AL.mult)
                    yield
                    uu = dp.tile([128, 1024], F, tag="dots")
                    for nt in range(2):
                        nc.tensor.matmul(uu[0:32, nt * 512:(nt + 1) * 512],
                                         tfold[:],
                                         tt[:, nt * 512:(nt + 1) * 512],
                                         start=True, stop=True)
                    nc.vector.tensor_copy(us[:, b * 1024:(b + 1) * 1024], uu[0:32, :])
                    yield

                # LN stats: mu row per bank
                sts = epi1.tile([1, 2048], R, tag="sts")
                for b in range(2):
                    st = dp.tile([128, 1024], F, tag="dots")
                    for nt in range(2):
                        nc.tensor.matmul(st[0:1, nt * 512:(nt + 1) * 512],
                                         tsmu[:, 0:1],
                                         us[:, b * 1024 + nt * 512:b * 1024 + (nt + 1) * 512],
                                         start=True, stop=True)
                    nc.vector.tensor_copy(sts[:, b * 1024:(b + 1) * 1024], st[0:1, :])
                    yield

                sq = epi1.tile([32, 2048], R, tag="sq")
                nc.vector.tensor_tensor(sq[:], us[:].bitcast(F), us[:].bitcast(F), AL.mult)
                # Bmu broadcast can run as soon as sts is ready (parallel with
                # the m2 / rsqrt chain)
                bmus = epi1.tile([32, 2048], F, tag="bmus")
                for b in range(2):
                    bm = dp.tile([128, 1024], F, tag="dots")
                    for nt in range(2):
                        nc.tensor.matmul(bm[0:32, nt * 512:(nt + 1) * 512], tones32[:],
                                         sts[:, b * 1024 + nt * 512:b * 1024 + (nt + 1) * 512],
                                         start=True, stop=True)
                    nc.vector.tensor_copy(bmus[:, b * 1024:(b + 1) * 1024], bm[0:32, :])
                yield
                # m2 per bank, consumed straight from PSUM for var
                msq = epi1.tile([1, 2048], F, tag="msq")
                nc.scalar.activation(msq[:], sts[:].bitcast(F), AF.Square)
                for b in range(2):
                    st = dp.tile([128, 1024], F, tag="dots")
                    for nt in range(2):
                        nc.tensor.matmul(st[0:1, nt * 512:(nt + 1) * 512],
                                         tsmu[:, 0:1],
                                         sq[:, b * 1024 + nt * 512:b * 1024 + (nt + 1) * 512],
                                         start=True, stop=True)
                    nc.vector.tensor_tensor(msq[:, b * 1024:(b + 1) * 1024], st[0:1, :],
                                            msq[:, b * 1024:(b + 1) * 1024], AL.subtract)
                    yield

                # rs = exp(-0.5*ln(var+eps))
                nc.scalar.activation(msq[:], msq[:], AF.Ln, bias=teps[:])
                rs = epi1.tile([1, 2048], R, tag="rs")
                nc.scalar.activation(rs[:], msq[:], AF.Exp, scale=-0.5)
                yield

                # normalize + gate: gt = ((us - Bmu)*Brs)*gam + bet, * sig
                gt = epi1.tile([32, 2048], R, tag="gt")
                for b in range(2):
                    brs = dp.tile([128, 1024], F, tag="dots")
                    for nt in range(2):
                        nc.tensor.matmul(brs[0:32, nt * 512:(nt + 1) * 512], tones32[:],
                                         rs[:, b * 1024 + nt * 512:b * 1024 + (nt + 1) * 512],
                                         start=True, stop=True)
                    t1 = epi1.tile([32, 1024], F, tag="t1")
                    nc.vector.tensor_tensor(t1[:], us[:, b * 1024:(b + 1) * 1024].bitcast(F), bmus[:, b * 1024:(b + 1) * 1024], AL.subtract)
                    nc.vector.tensor_tensor(t1[:], t1[:], brs[0:32, :], AL.mult)
                    nc.vector.tensor_scalar(t1[:], t1[:], tgam[:], tbet[:], AL.mult, AL.add)
                    nc.vector.tensor_tensor(gt[:, b * 1024:(b + 1) * 1024], t1[:],
                                            sgp[:, (2 * ip + b) * 1024:(2 * ip + b + 1) * 1024], AL.mult)
                    yield

                # output projection: yT[o, i] partials
                for oh in range(2):
                    yp = dp.tile([128, 1024], F, tag="dots")
                    for b in range(2):
                        for nt in range(2):
                            nc.tensor.matmul(yp[:, nt * 512:(nt + 1) * 512],
                                             twout[:, b, oh * 128:(oh + 1) * 128],
                                             gt[:, b * 1024 + nt * 512:b * 1024 + (nt + 1) * 512],
                                             start=(b == 0), stop=(b == 1))
                    ys = epi.tile([128, 1024], F, tag="ys")
                    nc.vector.tensor_copy(ys[:], yp[:])
                    nc.sync.dma_start(out=yT[oh * 128:(oh + 1) * 128, i0:i0 + 1024], in_=ys[:])
                    yield

            attention(0)
            # gates directly bank-packed: sgp [32, 4096], block (ip, b) at col
            # 1024*(2*ip+b).  sig = exp(-ln(exp(-(g+bg)) + 1))
            sgp = pers.tile([32, 4096], F, tag="sgp")
            for ip in range(2):
                for b in range(2):
                    ps = dp.tile([128, 1024], F, tag="dots")
                    for nt in range(2):
                        for f in range(2):
                            nc.tensor.matmul(
                                ps[0:32, nt * 512:(nt + 1) * 512],
                                twg[:, f, 32 * b:32 * b + 32],
                                xt[:, f, ip * 1024 + nt * 512:ip * 1024 + (nt + 1) * 512],
                                start=(f == 0), stop=(f == 1))
                    c0 = (2 * ip + b) * 1024
                    nc.scalar.activation(sgp[:, c0:c0 + 1024], ps[0:32, :], AF.Exp,
                                         scale=-1.0, bias=tnbg[:, b:b + 1])
            nc.scalar.activation(sgp[:], sgp[:], AF.Ln, bias=1.0)
            nc.scalar.activation(sgp[:], sgp[:], AF.Exp, scale=-1.0)

            gen0 = epilogue(0)
            attention(1, interleave=gen0)
            for _ in gen0:
                pass
            for _ in epilogue(1):
                pass

    nc.finalize()
    return nc


def _prep_core_inputs(inputs, bi, hg, lam):
    scale = DH ** -0.5
    x = np.asarray(inputs["x"], np.float32)
    Wq = np.asarray(inputs["Wq"], np.float32)
    Wkv = np.asarray(inputs["Wkv"], np.float32)
    Wout = np.asarray(inputs["Wout"], np.float32)
    Wg = np.asarray(inputs["Wg"], np.float32)
    bg = np.asarray(inputs["bg"], np.float32)
    g_ = np.asarray(inputs["ln_gamma"], np.float32)
    b_ = np.asarray(inputs["ln_beta"], np.float32)
    li = np.float32(1.0 - LAMBDA_INIT)

    c0 = 64 * hg
    wq_c = Wq[:, c0:c0 + 64] * scale
    wk_c = Wkv[:, c0:c0 + 64]
    wv_c = Wkv[:, 256 + c0:256 + c0 + 64]
    wg_c = Wg[:, c0:c0 + 64]
    wout_c = Wout[c0:c0 + 64, :]

    wqp = np.zeros((256, 128), np.float32)
    wkp = np.zeros((256, 128), np.float32)
    for d in range(4):
        wqp[:, 32 * d:32 * d + 16] = wq_c[:, 16 * d:16 * d + 16]
        wkp[:, 32 * d:32 * d + 16] = wk_c[:, 16 * d:16 * d + 16]

    woutp = np.zeros((32, 2, 256), np.float32)
    woutp[:, 0, :] = wout_c[0:32, :]
    woutp[:, 1, :] = wout_c[32:64, :]

    sel = np.zeros((4, 256), np.float32)
    sel[0, 0:32] = 1.0
    sel[1, 64:96] = lam
    sel[2, 128:160] = 1.0
    sel[3, 192:224] = lam

    fold = np.zeros((128, 32), np.float32)
    for r in range(32):
        fold[r, r] = 1.0
        fold[64 + r, r] = -1.0

    smu = np.zeros((32, 4), np.float32)
    smu[:, 0] = 1.0 / 32.0
    smu[:, 3] = 1.0 / 32.0

    ones32 = np.ones((1, 32), np.float32)
    onescol = np.zeros((128, 32), ml_dtypes.bfloat16)
    onescol[:, 0] = 1.0

    return {
        "xT": np.ascontiguousarray(x[bi].T),
        "wqp": wqp, "wkp": wkp,
        "wv": np.ascontiguousarray(wv_c),
        "wgc": np.ascontiguousarray(wg_c),
        "wout": woutp,
        "sel": sel, "fold": fold, "smu": smu,
        "ones32": ones32, "onescol": onescol,
        "epsc": np.full((1, 1), LN_EPS, np.float32),
        "gam": (g_[0:32] * li).reshape(32, 1).astype(np.float32),
        "bet": (b_[0:32] * li).reshape(32, 1).astype(np.float32),
        "nbg": (-bg[c0:c0 + 64]).reshape(64, 1).astype(np.float32),
    }


def kernel(**inputs) -> np.ndarray:
    lq1 = np.asarray(inputs["lq1"], np.float64)
    lk1 = np.asarray(inputs["lk1"], np.float64)
    lq2 = np.asarray(inputs["lq2"], np.float64)
    lk2 = np.asarray(inputs["lk2"], np.float64)
    lam = float(np.exp(np.sum(lq1 * lk1)) - np.exp(np.sum(lq2 * lk2)) + LAMBDA_INIT)
    bout = np.asarray(inputs["bout"], np.float32)

    if "nc" not in _cached:
        _cached["nc"] = build_kernel()
    nc = _cached["nc"]

    in_maps = []
    for c in range(NC):
        bi, hg = c // 4, c % 4
        in_maps.append(_prep_core_inputs(inputs, bi, hg, lam))

    import os
    trace = bool(int(os.environ.get("BASS_KERNEL_TRACE", "0")))
    res = run_bass_kernel_spmd(nc, in_maps, list(range(NC)), trace=trace)
    _cached["exec_time_ns"] = res.exec_time_ns
    _cached["trace"] = res.instructions_and_trace
    out = np.zeros((B, N, DIM), np.float32)
    for c in range(NC):
        bi = c // 4
        out[bi] += res.results[c]["yT"].T
    out += bout
    return out


# revision 38
# speedup vs baseline: 1.0239x; 1.0239x over previous
"""DiffAttn2d TRN2 Bass kernel.

Sharding: 8 cores = 2 (batch) x 4 (head-groups of 2 heads / 4 doubled-heads).
Per core, everything is computed channel-major (scores transposed: [key, query])
so softmax needs no transposes:
  - dots^T via row-packed K=16 fp32r matmuls (4 doubled heads in 4 PE row groups)
  - one ACT exp pass PSUM->SBUF (bf16), the only O(n^2) elementwise op
  - attn @ v as Z^T = [v | ones]^T @ exp^T (bf16, col-block packed 2 dheads/matmul);
    the ones column yields the softmax denominators for free
  - u = Z0/s0 - lam*Z1/s1 via reciprocal rows (exp(-ln(s))) broadcast with
    tiny selector matmuls, folded with a +/-1 matmul
  - LayerNorm stats via ones-matmuls; rsqrt via exp(-0.5*ln(var+eps))
  - sigmoid gating via exp/ln (same ACT table set as exp: zero table switches)
  - y^T = Wout^T @ gated, row-parallel partials summed on host (+ bout)
"""
import sys
sys.path.insert(0, "/opt/trn_rl_repo")

import math
import numpy as np
import ml_dtypes

import concourse.bass as bass
import concourse.bacc as bacc_mod
import concourse.mybir as mybir
from concourse.tile import TileContext
from concourse.bass_utils import run_bass_kernel_spmd

F = mybir.dt.float32
R = mybir.dt.float32r
BF = mybir.dt.bfloat16
AF = mybir.ActivationFunctionType
AL = mybir.AluOpType

H, DH = 8, 16
DEPTH = 1
LAMBDA_INIT = 0.8 - 0.6 * math.exp(-0.3 * DEPTH)
LN_EPS = 1e-5
B, N, DIM = 2, 2048, 256
NC = 8

_cached = {}


def build_kernel():
    nc = bacc_mod.Bacc()
    xT = nc.declare_dram_parameter("xT", [DIM, N], R, isOutput=False)
    wqp = nc.declare_dram_parameter("wqp", [DIM, 128], R, isOutput=False)
    wkp = nc.declare_dram_parameter("wkp", [DIM, 128], R, isOutput=False)
    wv = nc.declare_dram_parameter("wv", [DIM, 64], R, isOutput=False)
    wgc = nc.declare_dram_parameter("wgc", [DIM, 64], R, isOutput=False)
    wout = nc.declare_dram_parameter("wout", [32, 2, 256], R, isOutput=False)  # [ch, bank, out]
    sel = nc.declare_dram_parameter("sel", [4, 256], R, isOutput=False)        # [:, 0:128]=A, [:,128:]=B
    fold = nc.declare_dram_parameter("fold", [128, 32], R, isOutput=False)
    smu = nc.declare_dram_parameter("smu", [32, 4], R, isOutput=False)         # cols 0:2 mu-pass, 2:4 m2-pass
    ones32 = nc.declare_dram_parameter("ones32", [1, 32], R, isOutput=False)
    onescol = nc.declare_dram_parameter("onescol", [128, 32], BF, isOutput=False)
    gam = nc.declare_dram_parameter("gam", [32, 1], F, isOutput=False)
    bet = nc.declare_dram_parameter("bet", [32, 1], F, isOutput=False)
    nbg = nc.declare_dram_parameter("nbg", [32, 2], F, isOutput=False)
    epsc = nc.declare_dram_parameter("epsc", [1, 1], F, isOutput=False)
    yT = nc.declare_dram_parameter("yT", [DIM, N], F, isOutput=True)

    with TileContext(nc) as tc:
        with tc.tile_pool(name="pers", bufs=1) as pers, \
             tc.tile_pool(name="work", bufs=6) as work, \
             tc.tile_pool(name="epi", bufs=2) as epi, \
             tc.tile_pool(name="epi1", bufs=1) as epi1, \
             tc.tile_pool(name="zsp", bufs=4) as zsp, \
             tc.tile_pool(name="dp", bufs=2, space="PSUM") as dp, \
             tc.tile_pool(name="zp", bufs=2, space="PSUM") as zp:

            # ---------------- load persistent data ----------------
            xt = pers.tile([128, 2, N], R, tag="xt")      # [:, f, :] feature chunk f
            xTr = xT.rearrange("(f p) n -> p f n", p=128)
            for f in range(2):
                nc.sync.dma_start(out=xt[:, f, :], in_=xTr[:, f, :])
            twkp = pers.tile([128, 2, 128], R, tag="twkp")
            nc.sync.dma_start(out=twkp[:], in_=wkp.rearrange("(f p) m -> p f m", p=128))
            twqp = pers.tile([128, 2, 128], R, tag="twqp")
            nc.sync.dma_start(out=twqp[:], in_=wqp.rearrange("(f p) m -> p f m", p=128))
            twv = pers.tile([128, 2, 64], R, tag="twv")
            nc.sync.dma_start(out=twv[:], in_=wv.rearrange("(f p) m -> p f m", p=128))
            tones = pers.tile([128, 32], BF, tag="tones")
            nc.sync.dma_start(out=tones[:], in_=onescol[:])
            twg = pers.tile([128, 2, 64], R, tag="twg")
            nc.sync.dma_start(out=twg[:], in_=wgc.rearrange("(f p) m -> p f m", p=128))
            twout = pers.tile([32, 2, 256], R, tag="twout")
            nc.sync.dma_start(out=twout[:], in_=wout[:])
            tsel = pers.tile([4, 256], R, tag="tsel")
            nc.sync.dma_start(out=tsel[:], in_=sel[:])
            tfold = pers.tile([128, 32], R, tag="tfold")
            nc.sync.dma_start(out=tfold[:], in_=fold[:])
            tsmu = pers.tile([32, 4], R, tag="tsmu")
            nc.sync.dma_start(out=tsmu[:], in_=smu[:])
            tones32 = pers.tile([1, 32], R, tag="tones32")
            nc.sync.dma_start(out=tones32[:], in_=ones32[:])
            tgam = pers.tile([32, 1], F, tag="tgam")
            nc.sync.dma_start(out=tgam[:], in_=gam[:])
            tbet = pers.tile([32, 1], F, tag="tbet")
            nc.sync.dma_start(out=tbet[:], in_=bet[:])
            tnbg = pers.tile([32, 2], F, tag="tnbg")
            nc.sync.dma_start(out=tnbg[:], in_=nbg[:])
            teps = pers.tile([1, 1], F, tag="teps")
            nc.sync.dma_start(out=teps[:], in_=epsc[:])

            # ---------------- projections ----------------
            # q^T / k^T packed: partition 32d+j (j<16) = channel j of dhead d
            qTp = pers.tile([128, N], R, tag="qTp")
            kTp = pers.tile([128, N], R, tag="kTp")
            vpp = pers.tile([128, 16, 128], BF, tag="vpp")

            def proj_qk(dst, w, it):
                ps = dp.tile([128, 1024], F, tag="dots")
                for f in range(2):
                    nc.tensor.matmul(ps[:, 0:512], w[:, f, :], xt[:, f, it * 512:(it + 1) * 512],
                                     start=(f == 0), stop=(f == 1))
                nc.vector.tensor_copy(dst[:, it * 512:(it + 1) * 512], ps[:, 0:512])

            def proj_v(jc):
                ps = dp.tile([128, 1024], F, tag="dots")
                for f in range(2):
                    nc.tensor.matmul(ps[:, 0:64], xt[:, f, jc * 128:(jc + 1) * 128], twv[:, f, :],
                                     start=(f == 0), stop=(f == 1))
                nc.vector.tensor_copy(vpp[:, jc, 0:32], ps[:, 0:32])
                nc.vector.tensor_copy(vpp[:, jc, 64:96], ps[:, 32:64])
                nc.vector.tensor_copy(vpp[:, jc, 32:64], tones[:])
                nc.vector.tensor_copy(vpp[:, jc, 96:128], tones[:])

            proj_qk(kTp, twkp, 0)
            proj_qk(qTp, twqp, 0)
            proj_qk(qTp, twqp, 1)
            for jc in range(4):
                proj_v(jc)
            for it in range(1, 4):
                proj_qk(kTp, twkp, it)
            proj_qk(qTp, twqp, 2)
            proj_qk(qTp, twqp, 3)
            for jc in range(4, 16):
                proj_v(jc)

            # ---------------- main: attention ----------------
            saved = {}

            def attention(ip, interleave=None):
                i0 = ip * 1024
                zA = zp.tile([128, 1024], F, tag="z")
                zB = zp.tile([128, 1024], F, tag="z")
                zbank = (zA, zA, zB, zB)
                for jc in range(16):
                    if interleave is not None and jc >= 2:
                        next(interleave, None)
                    j0 = jc * 128
                    for pair in range(2):
                        ebfs = []
                        dts = []
                        for dd in range(2):
                            d = 2 * pair + dd
                            r0 = 32 * d
                            dt_ = dp.tile([128, 1024], F, tag="dots")
                            for h in range(2):
                                nc.tensor.matmul(
                                    dt_[:, h * 512:(h + 1) * 512],
                                    kTp[r0:r0 + 16, j0:j0 + 128],
                                    qTp[r0:r0 + 16, i0 + h * 512:i0 + (h + 1) * 512],
                                    start=True, stop=True,
                                    tile_position=(r0, 0),
                                )
                            eb = work.tile([128, 1024], BF, tag="ebf")
                            nc.scalar.activation(eb[:], dt_[:], AF.Exp)
                            ebfs.append(eb)
                            dts.append(dt_)
                        for dd in range(2):
                            d = 2 * pair + dd
                            zt = zbank[d]
                            for h in range(2):
                                nc.tensor.matmul(
                                    zt[64 * dd:64 * dd + 64, h * 512:(h + 1) * 512],
                                    vpp[:, jc, 64 * pair:64 * pair + 64],
                                    ebfs[dd][:, h * 512:(h + 1) * 512],
                                    start=(jc == 0), stop=(jc == 15),
                                    tile_position=(0, 64 * dd),
                                    skip_group_check=True,
                                )

                # hand Z off to SBUF immediately so the Z psum slots free up
                zsA = zsp.tile([128, 1024], F, tag="zs")
                zsB = zsp.tile([128, 1024], F, tag="zs")
                nc.vector.tensor_copy(zsA[:], zA[:])
                nc.vector.tensor_copy(zsB[:], zB[:])
                srows = epi.tile([4, 1024], F, tag="srows")
                nc.sync.dma_start(out=srows[0:2, :], in_=zsA[32:128:64, :])
                nc.sync.dma_start(out=srows[2:4, :], in_=zsB[32:128:64, :])
                saved[ip] = (zsA, zsB, srows)

            def epilogue(ip):
                i0 = ip * 1024
                zsA, zsB, srows = saved[ip]
                rinv = epi1.tile([4, 1024], R, tag="rinv")
                nc.scalar.activation(srows[:], srows[:], AF.Ln)
                nc.scalar.activation(rinv[:], srows[:], AF.Exp, scale=-1.0)
                yield

                us = [epi1.tile([32, 1024], R, tag=f"us{b}", name=f"us{b}") for b in range(2)]
                sts = [epi1.tile([1, 1024], R, tag=f"sts{b}", name=f"sts{b}") for b in range(2)]
                sq = [epi1.tile([32, 1024], R, tag=f"sq{b}", name=f"sq{b}") for b in range(2)]
                bmus = [epi1.tile([32, 1024], F, tag=f"bmus{b}", name=f"bmus{b}") for b in range(2)]
                msq = [epi1.tile([1, 1024], F, tag=f"msq{b}", name=f"msq{b}") for b in range(2)]
                rs = [epi1.tile([1, 1024], R, tag=f"rs{b}", name=f"rs{b}") for b in range(2)]
                gt = [epi1.tile([32, 1024], R, tag=f"gt{b}", name=f"gt{b}") for b in range(2)]

                tts = []
                for b, zs in ((0, zsA), (1, zsB)):
                    bf_ = dp.tile([128, 1024], F, tag="dots")
                    for nt in range(2):
                        nc.tensor.matmul(bf_[:, nt * 512:(nt + 1) * 512],
                                         tsel[:, 128 * b:128 * (b + 1)],
                                         rinv[:, nt * 512:(nt + 1) * 512],
                                         start=True, stop=True)
                    tt = epi1.tile([128, 1024], R, tag=f"tt{b}", name=f"tt{b}")
                    nc.vector.tensor_tensor(tt[:], zs[:], bf_[:], AL.mult)
                    tts.append(tt)
                    yield
                for b in range(2):
                    uu = dp.tile([128, 1024], F, tag="dots")
                    for nt in range(2):
                        nc.tensor.matmul(uu[0:32, nt * 512:(nt + 1) * 512],
                                         tfold[:],
                                         tts[b][:, nt * 512:(nt + 1) * 512],
                                         start=True, stop=True)
                    nc.vector.tensor_copy(us[b][:], uu[0:32, :])
                    yield

                for b in range(2):
                    st = dp.tile([128, 1024], F, tag="dots")
                    for nt in range(2):
                        nc.tensor.matmul(st[0:1, nt * 512:(nt + 1) * 512],
                                         tsmu[:, 0:1],
                                         us[b][:, nt * 512:(nt + 1) * 512],
                                         start=True, stop=True)
                    nc.vector.tensor_copy(sts[b][:], st[0:1, :])
                    nc.vector.tensor_tensor(sq[b][:], us[b][:].bitcast(F), us[b][:].bitcast(F), AL.mult)
                    yield
                for b in range(2):
                    bm = dp.tile([128, 1024], F, tag="dots")
                    for nt in range(2):
                        nc.tensor.matmul(bm[0:32, nt * 512:(nt + 1) * 512], tones32[:],
                                         sts[b][:, nt * 512:(nt + 1) * 512],
                                         start=True, stop=True)
                    nc.vector.tensor_copy(bmus[b][:], bm[0:32, :])
                    nc.scalar.activation(msq[b][:], sts[b][:].bitcast(F), AF.Square)
                    yield
                for b in range(2):
                    st2 = dp.tile([128, 1024], F, tag="dots")
                    for nt in range(2):
                        nc.tensor.matmul(st2[0:1, nt * 512:(nt + 1) * 512],
                                         tsmu[:, 0:1],
                                         sq[b][:, nt * 512:(nt + 1) * 512],
                                         start=True, stop=True)
                    nc.vector.tensor_tensor(msq[b][:], st2[0:1, :], msq[b][:], AL.subtract)
                    nc.scalar.activation(msq[b][:], msq[b][:], AF.Ln, bias=teps[:])
                    nc.scalar.activation(rs[b][:], msq[b][:], AF.Exp, scale=-0.5)
                    yield
                for b in range(2):
                    brs = dp.tile([128, 1024], F, tag="dots")
                    for nt in range(2):
                        nc.tensor.matmul(brs[0:32, nt * 512:(nt + 1) * 512], tones32[:],
                                         rs[b][:, nt * 512:(nt + 1) * 512],
                                         start=True, stop=True)
                    t1 = epi1.tile([32, 1024], F, tag=f"t1{b}", name=f"t1{b}")
                    nc.vector.tensor_tensor(t1[:], us[b][:].bitcast(F), bmus[b][:], AL.subtract)
                    nc.vector.tensor_tensor(t1[:], t1[:], brs[0:32, :], AL.mult)
                    nc.vector.tensor_scalar(t1[:], t1[:], tgam[:], tbet[:], AL.mult, AL.add)
                    nc.vector.tensor_tensor(gt[b][:], t1[:],
                                            sgp[:, (2 * ip + b) * 1024:(2 * ip + b + 1) * 1024], AL.mult)
                    yield

                # output projection: yT[o, i] partials
                for oh in range(2):
                    yp = dp.tile([128, 1024], F, tag="dots")
                    for b in range(2):
                        for nt in range(2):
                            nc.tensor.matmul(yp[:, nt * 512:(nt + 1) * 512],
                                             twout[:, b, oh * 128:(oh + 1) * 128],
                                             gt[b][:, nt * 512:(nt + 1) * 512],
                                             start=(b == 0), stop=(b == 1))
                    ys = epi.tile([128, 1024], F, tag="ys")
                    nc.vector.tensor_copy(ys[:], yp[:])
                    nc.sync.dma_start(out=yT[oh * 128:(oh + 1) * 128, i0:i0 + 1024], in_=ys[:])
                    yield

            attention(0)
            # gates directly bank-packed: sgp [32, 4096], block (ip, b) at col
            # 1024*(2*ip+b).  sig = exp(-ln(exp(-(g+bg)) + 1))
            sgp = pers.tile([32, 4096], F, tag="sgp")
            for ip in range(2):
                for b in range(2):
                    ps = dp.tile([128, 1024], F, tag="dots")
                    for nt in range(2):
                        for f in range(2):
                            nc.tensor.matmul(
                                ps[0:32, nt * 512:(nt + 1) * 512],
                                twg[:, f, 32 * b:32 * b + 32],
                                xt[:, f, ip * 1024 + nt * 512:ip * 1024 + (nt + 1) * 512],
                                start=(f == 0), stop=(f == 1))
                    c0 = (2 * ip + b) * 1024
                    nc.scalar.activation(sgp[:, c0:c0 + 1024], ps[0:32, :], AF.Exp,
                                         scale=-1.0, bias=tnbg[:, b:b + 1])
            nc.scalar.activation(sgp[:], sgp[:], AF.Ln, bias=1.0)
            nc.scalar.activation(sgp[:], sgp[:], AF.Exp, scale=-1.0)

            gen0 = epilogue(0)
            attention(1, interleave=gen0)
            for _ in gen0:
                pass
            for _ in epilogue(1):
                pass

    nc.finalize()
    return nc


def _prep_core_inputs(inputs, bi, hg, lam):
    scale = DH ** -0.5
    x = np.asarray(inputs["x"], np.float32)
    Wq = np.asarray(inputs["Wq"], np.float32)
    Wkv = np.asarray(inputs["Wkv"], np.float32)
    Wout = np.asarray(inputs["Wout"], np.float32)
    Wg = np.asarray(inputs["Wg"], np.float32)
    bg = np.asarray(inputs["bg"], np.float32)
    g_ = np.asarray(inputs["ln_gamma"], np.float32)
    b_ = np.asarray(inputs["ln_beta"], np.float32)
    li = np.float32(1.0 - LAMBDA_INIT)

    c0 = 64 * hg
    wq_c = Wq[:, c0:c0 + 64] * scale
    wk_c = Wkv[:, c0:c0 + 64]
    wv_c = Wkv[:, 256 + c0:256 + c0 + 64]
    wg_c = Wg[:, c0:c0 + 64]
    wout_c = Wout[c0:c0 + 64, :]

    wqp = np.zeros((256, 128), np.float32)
    wkp = np.zeros((256, 128), np.float32)
    for d in range(4):
        wqp[:, 32 * d:32 * d + 16] = wq_c[:, 16 * d:16 * d + 16]
        wkp[:, 32 * d:32 * d + 16] = wk_c[:, 16 * d:16 * d + 16]

    woutp = np.zeros((32, 2, 256), np.float32)
    woutp[:, 0, :] = wout_c[0:32, :]
    woutp[:, 1, :] = wout_c[32:64, :]

    sel = np.zeros((4, 256), np.float32)
    sel[0, 0:32] = 1.0
    sel[1, 64:96] = lam
    sel[2, 128:160] = 1.0
    sel[3, 192:224] = lam

    fold = np.zeros((128, 32), np.float32)
    for r in range(32):
        fold[r, r] = 1.0
        fold[64 + r, r] = -1.0

    smu = np.zeros((32, 4), np.float32)
    smu[:, 0] = 1.0 / 32.0
    smu[:, 3] = 1.0 / 32.0

    ones32 = np.ones((1, 32), np.float32)
    onescol = np.zeros((128, 32), ml_dtypes.bfloat16)
    onescol[:, 0] = 1.0

    return {
        "xT": np.ascontiguousarray(x[bi].T),
        "wqp": wqp, "wkp": wkp,
        "wv": np.ascontiguousarray(wv_c),
        "wgc": np.ascontiguousarray(wg_c),
        "wout": woutp,
        "sel": sel, "fold": fold, "smu": smu,
        "ones32": ones32, "onescol": onescol,
        "epsc": np.full((1, 1), LN_EPS, np.float32),
        "gam": (g_[0:32] * li).reshape(32, 1).astype(np.float32),
        "bet": (b_[0:32] * li).reshape(32, 1).astype(np.float32),
        "nbg": (-bg[c0:c0 + 64]).reshape(64, 1).astype(np.float32),
    }


def kernel(**inputs) -> np.ndarray:
    lq1 = np.asarray(inputs["lq1"], np.float64)
    lk1 = np.asarray(inputs["lk1"], np.float64)
    lq2 = np.asarray(inputs["lq2"], np.float64)
    lk2 = np.asarray(inputs["lk2"], np.float64)
    lam = float(np.exp(np.sum(lq1 * lk1)) - np.exp(np.sum(lq2 * lk2)) + LAMBDA_INIT)
    bout = np.asarray(inputs["bout"], np.float32)

    if "nc" not in _cached:
        _cached["nc"] = build_kernel()
    nc = _cached["nc"]

    in_maps = []
    for c in range(NC):
        bi, hg = c // 4, c % 4
        in_maps.append(_prep_core_inputs(inputs, bi, hg, lam))

    import os
    trace = bool(int(os.environ.get("BASS_KERNEL_TRACE", "0")))
    res = run_bass_kernel_spmd(nc, in_maps, list(range(NC)), trace=trace)
    _cached["exec_time_ns"] = res.exec_time_ns
    _cached["trace"] = res.instructions_and_trace
    out = np.zeros((B, N, DIM), np.float32)
    for c in range(NC):
        bi = c // 4
        out[bi] += res.results[c]["yT"].T
    out += bout
    return out


# revision 45
# speedup vs baseline: 1.0483x; 1.0239x over previous
"""DiffAttn2d TRN2 Bass kernel.

Sharding: 8 cores = 2 (batch) x 4 (head-groups of 2 heads / 4 doubled-heads).
Per core, everything is computed channel-major (scores transposed: [key, query])
so softmax needs no transposes:
  - dots^T via row-packed K=16 fp32r matmuls (4 doubled heads in 4 PE row groups)
  - one ACT exp pass PSUM->SBUF (bf16), the only O(n^2) elementwise op
  - attn @ v as Z^T = [v | ones]^T @ exp^T (bf16, col-block packed 2 dheads/matmul);
    the ones column yields the softmax denominators for free
  - u = Z0/s0 - lam*Z1/s1 via reciprocal rows (exp(-ln(s))) broadcast with
    tiny selector matmuls, folded with a +/-1 matmul
  - LayerNorm stats via ones-matmuls; rsqrt via exp(-0.5*ln(var+eps))
  - sigmoid gating via exp/ln (same ACT table set as exp: zero table switches)
  - y^T = Wout^T @ gated, row-parallel partials summed on host (+ bout)
"""
import sys
sys.path.insert(0, "/opt/trn_rl_repo")

import math
import numpy as np
import ml_dtypes

import concourse.bass as bass
import concourse.bacc as bacc_mod
import concourse.mybir as mybir
from concourse.tile import TileContext
from concourse.bass_utils import run_bass_kernel_spmd

F = mybir.dt.float32
R = mybir.dt.float32r
BF = mybir.dt.bfloat16
AF = mybir.ActivationFunctionType
AL = mybir.AluOpType

H, DH = 8, 16
DEPTH = 1
LAMBDA_INIT = 0.8 - 0.6 * math.exp(-0.3 * DEPTH)
LN_EPS = 1e-5
B, N, DIM = 2, 2048, 256
NC = 8

_cached = {}


def build_kernel():
    nc = bacc_mod.Bacc()
    xT = nc.declare_dram_parameter("xT", [DIM, N], R, isOutput=False)
    wqp = nc.declare_dram_parameter("wqp", [DIM, 128], R, isOutput=False)
    wkp = nc.declare_dram_parameter("wkp", [DIM, 128], R, isOutput=False)
    wv = nc.declare_dram_parameter("wv", [DIM, 64], R, isOutput=False)
    wgc = nc.declare_dram_parameter("wgc", [DIM, 64], R, isOutput=False)
    wout = nc.declare_dram_parameter("wout", [32, 2, 256], R, isOutput=False)  # [ch, bank, out]
    sel = nc.declare_dram_parameter("sel", [4, 256], R, isOutput=False)        # [:, 0:128]=A, [:,128:]=B
    fold = nc.declare_dram_parameter("fold", [128, 32], R, isOutput=False)
    smu = nc.declare_dram_parameter("smu", [32, 4], R, isOutput=False)         # cols 0:2 mu-pass, 2:4 m2-pass
    ones32 = nc.declare_dram_parameter("ones32", [1, 32], R, isOutput=False)
    onescol = nc.declare_dram_parameter("onescol", [128, 32], BF, isOutput=False)
    gam = nc.declare_dram_parameter("gam", [32, 1], F, isOutput=False)
    bet = nc.declare_dram_parameter("bet", [32, 1], F, isOutput=False)
    nbg = nc.declare_dram_parameter("nbg", [32, 2], F, isOutput=False)
    epsc = nc.declare_dram_parameter("epsc", [1, 1], F, isOutput=False)
    yT = nc.declare_dram_parameter("yT", [DIM, N], F, isOutput=True)

    with TileContext(nc) as tc:
        with tc.tile_pool(name="pers", bufs=1) as pers, \
             tc.tile_pool(name="work", bufs=6) as work, \
             tc.tile_pool(name="epi", bufs=2) as epi, \
             tc.tile_pool(name="epi1", bufs=1) as epi1, \
             tc.tile_pool(name="zsp", bufs=4) as zsp, \
             tc.tile_pool(name="dp", bufs=2, space="PSUM") as dp, \
             tc.tile_pool(name="zp", bufs=2, space="PSUM") as zp:

            # ---------------- load persistent data ----------------
            xt = pers.tile([128, 2, N], R, tag="xt")      # [:, f, :] feature chunk f
            xTr = xT.rearrange("(f p) n -> p f n", p=128)
            for f in range(2):
                nc.sync.dma_start(out=xt[:, f, :], in_=xTr[:, f, :])
            twkp = pers.tile([128, 2, 128], R, tag="twkp")
            nc.sync.dma_start(out=twkp[:], in_=wkp.rearrange("(f p) m -> p f m", p=128))
            twqp = pers.tile([128, 2, 128], R, tag="twqp")
            nc.sync.dma_start(out=twqp[:], in_=wqp.rearrange("(f p) m -> p f m", p=128))
            twv = pers.tile([128, 2, 64], R, tag="twv")
            nc.sync.dma_start(out=twv[:], in_=wv.rearrange("(f p) m -> p f m", p=128))
            tones = pers.tile([128, 32], BF, tag="tones")
            nc.sync.dma_start(out=tones[:], in_=onescol[:])
            twg = pers.tile([128, 2, 64], R, tag="twg")
            nc.sync.dma_start(out=twg[:], in_=wgc.rearrange("(f p) m -> p f m", p=128))
            twout = pers.tile([32, 2, 256], R, tag="twout")
            nc.sync.dma_start(out=twout[:], in_=wout[:])
            tsel = pers.tile([4, 256], R, tag="tsel")
            nc.sync.dma_start(out=tsel[:], in_=sel[:])
            tfold = pers.tile([128, 32], R, tag="tfold")
            nc.sync.dma_start(out=tfold[:], in_=fold[:])
            tsmu = pers.tile([32, 4], R, tag="tsmu")
            nc.sync.dma_start(out=tsmu[:], in_=smu[:])
            tones32 = pers.tile([1, 32], R, tag="tones32")
            nc.sync.dma_start(out=tones32[:], in_=ones32[:])
            tgam = pers.tile([32, 1], F, tag="tgam")
            nc.sync.dma_start(out=tgam[:], in_=gam[:])
            tbet = pers.tile([32, 1], F, tag="tbet")
            nc.sync.dma_start(out=tbet[:], in_=bet[:])
            tnbg = pers.tile([32, 2], F, tag="tnbg")
            nc.sync.dma_start(out=tnbg[:], in_=nbg[:])
            teps = pers.tile([1, 1], F, tag="teps")
            nc.sync.dma_start(out=teps[:], in_=epsc[:])

            # ---------------- projections ----------------
            # q^T / k^T packed: partition 32d+j (j<16) = channel j of dhead d
            qTp = pers.tile([128, N], R, tag="qTp")
            kTp = pers.tile([128, N], R, tag="kTp")
            vpp = pers.tile([128, 16, 128], BF, tag="vpp")

            def proj_qk(dst, w, it):
                ps = dp.tile([128, 1024], F, tag="dots")
                for f in range(2):
                    nc.tensor.matmul(ps[:, 0:512], w[:, f, :], xt[:, f, it * 512:(it + 1) * 512],
                                     start=(f == 0), stop=(f == 1))
                nc.vector.tensor_copy(dst[:, it * 512:(it + 1) * 512], ps[:, 0:512])

            def proj_v(jc):
                ps = dp.tile([128, 1024], F, tag="dots")
                for f in range(2):
                    nc.tensor.matmul(ps[:, 0:64], xt[:, f, jc * 128:(jc + 1) * 128], twv[:, f, :],
                                     start=(f == 0), stop=(f == 1))
                nc.vector.tensor_copy(vpp[:, jc, 0:32], ps[:, 0:32])
                nc.vector.tensor_copy(vpp[:, jc, 64:96], ps[:, 32:64])
                nc.vector.tensor_copy(vpp[:, jc, 32:64], tones[:])
                nc.vector.tensor_copy(vpp[:, jc, 96:128], tones[:])

            proj_qk(kTp, twkp, 0)
            proj_qk(qTp, twqp, 0)
            proj_qk(qTp, twqp, 1)
            for jc in range(4):
                proj_v(jc)
            for it in range(1, 4):
                proj_qk(kTp, twkp, it)
            proj_qk(qTp, twqp, 2)
            proj_qk(qTp, twqp, 3)
            for jc in range(4, 16):
                proj_v(jc)

            # ---------------- main: attention ----------------
            saved = {}

            def attention(ip, interleave=None):
                i0 = ip * 1024
                zA = zp.tile([128, 1024], F, tag="z")
                zB = zp.tile([128, 1024], F, tag="z")
                zbank = (zA, zA, zB, zB)
                for jc in range(16):
                    if interleave is not None and jc >= 7:
                        next(interleave, None)
                    j0 = jc * 128
                    for pair in range(2):
                        ebfs = []
                        dts = []
                        for dd in range(2):
                            d = 2 * pair + dd
                            r0 = 32 * d
                            dt_ = dp.tile([128, 1024], F, tag="dots")
                            for h in range(2):
                                nc.tensor.matmul(
                                    dt_[:, h * 512:(h + 1) * 512],
                                    kTp[r0:r0 + 16, j0:j0 + 128],
                                    qTp[r0:r0 + 16, i0 + h * 512:i0 + (h + 1) * 512],
                                    start=True, stop=True,
                                    tile_position=(r0, 0),
                                )
                            eb = work.tile([128, 1024], BF, tag="ebf")
                            nc.scalar.activation(eb[:], dt_[:], AF.Exp)
                            ebfs.append(eb)
                            dts.append(dt_)
                        for dd in range(2):
                            d = 2 * pair + dd
                            zt = zbank[d]
                            for h in range(2):
                                nc.tensor.matmul(
                                    zt[64 * dd:64 * dd + 64, h * 512:(h + 1) * 512],
                                    vpp[:, jc, 64 * pair:64 * pair + 64],
                                    ebfs[dd][:, h * 512:(h + 1) * 512],
                                    start=(jc == 0), stop=(jc == 15),
                                    tile_position=(0, 64 * dd),
                                    skip_group_check=True,
                                )

                # hand Z off to SBUF immediately so the Z psum slots free up
                zsA = zsp.tile([128, 1024], F, tag="zs")
                zsB = zsp.tile([128, 1024], F, tag="zs")
                nc.vector.tensor_copy(zsA[:], zA[:])
                nc.vector.tensor_copy(zsB[:], zB[:])
                srows = epi.tile([4, 1024], F, tag="srows")
                nc.sync.dma_start(out=srows[0:2, :], in_=zsA[32:128:64, :])
                nc.sync.dma_start(out=srows[2:4, :], in_=zsB[32:128:64, :])
                saved[ip] = (zsA, zsB, srows)

            def epilogue(ip):
                i0 = ip * 1024
                zsA, zsB, srows = saved[ip]
                rinv = epi1.tile([4, 1024], R, tag="rinv")
                nc.scalar.activation(srows[:], srows[:], AF.Ln)
                nc.scalar.activation(rinv[:], srows[:], AF.Exp, scale=-1.0)
                yield

                us = [epi1.tile([32, 1024], R, tag=f"us{b}", name=f"us{b}") for b in range(2)]
                sts = [epi1.tile([1, 1024], R, tag=f"sts{b}", name=f"sts{b}") for b in range(2)]
                sq = [epi1.tile([32, 1024], R, tag=f"sq{b}", name=f"sq{b}") for b in range(2)]
                bmus = [epi1.tile([32, 1024], F, tag=f"bmus{b}", name=f"bmus{b}") for b in range(2)]
                msq = [epi1.tile([1, 1024], F, tag=f"msq{b}", name=f"msq{b}") for b in range(2)]
                rs = [epi1.tile([1, 1024], R, tag=f"rs{b}", name=f"rs{b}") for b in range(2)]
                gt = [epi1.tile([32, 1024], R, tag=f"gt{b}", name=f"gt{b}") for b in range(2)]

                tts = []
                for b, zs in ((0, zsA), (1, zsB)):
                    bf_ = dp.tile([128, 1024], F, tag="dots")
                    for nt in range(2):
                        nc.tensor.matmul(bf_[:, nt * 512:(nt + 1) * 512],
                                         tsel[:, 128 * b:128 * (b + 1)],
                                         rinv[:, nt * 512:(nt + 1) * 512],
                                         start=True, stop=True)
                    tt = epi1.tile([128, 1024], R, tag=f"tt{b}", name=f"tt{b}")
                    nc.vector.tensor_tensor(tt[:], zs[:], bf_[:], AL.mult)
                    tts.append(tt)
                    yield
                for b in range(2):
                    uu = dp.tile([128, 1024], F, tag="dots")
                    for nt in range(2):
                        nc.tensor.matmul(uu[0:32, nt * 512:(nt + 1) * 512],
                                         tfold[:],
                                         tts[b][:, nt * 512:(nt + 1) * 512],
                                         start=True, stop=True)
                    nc.vector.tensor_copy(us[b][:], uu[0:32, :])
                    yield

                for b in range(2):
                    st = dp.tile([128, 1024], F, tag="dots")
                    for nt in range(2):
                        nc.tensor.matmul(st[0:1, nt * 512:(nt + 1) * 512],
                                         tsmu[:, 0:1],
                                         us[b][:, nt * 512:(nt + 1) * 512],
                                         start=True, stop=True)
                    nc.vector.tensor_copy(sts[b][:], st[0:1, :])
                    nc.vector.tensor_tensor(sq[b][:], us[b][:].bitcast(F), us[b][:].bitcast(F), AL.mult)
                    yield
                for b in range(2):
                    bm = dp.tile([128, 1024], F, tag="dots")
                    for nt in range(2):
                        nc.tensor.matmul(bm[0:32, nt * 512:(nt + 1) * 512], tones32[:],
                                         sts[b][:, nt * 512:(nt + 1) * 512],
                                         start=True, stop=True)
                    nc.vector.tensor_copy(bmus[b][:], bm[0:32, :])
                    nc.scalar.activation(msq[b][:], sts[b][:].bitcast(F), AF.Square)
                    yield
                for b in range(2):
                    st2 = dp.tile([128, 1024], F, tag="dots")
                    for nt in range(2):
                        nc.tensor.matmul(st2[0:1, nt * 512:(nt + 1) * 512],
                                         tsmu[:, 0:1],
                                         sq[b][:, nt * 512:(nt + 1) * 512],
                                         start=True, stop=True)
                    nc.vector.tensor_tensor(msq[b][:], st2[0:1, :], msq[b][:], AL.subtract)
                    nc.scalar.activation(msq[b][:], msq[b][:], AF.Ln, bias=teps[:])
                    nc.scalar.activation(rs[b][:], msq[b][:], AF.Exp, scale=-0.5)
                    yield
                for b in range(2):
                    brs = dp.tile([128, 1024], F, tag="dots")
                    for nt in range(2):
                        nc.tensor.matmul(brs[0:32, nt * 512:(nt + 1) * 512], tones32[:],
                                         rs[b][:, nt * 512:(nt + 1) * 512],
                                         start=True, stop=True)
                    t1 = epi1.tile([32, 1024], F, tag=f"t1{b}", name=f"t1{b}")
                    nc.vector.tensor_tensor(t1[:], us[b][:].bitcast(F), bmus[b][:], AL.subtract)
                    nc.vector.tensor_tensor(t1[:], t1[:], brs[0:32, :], AL.mult)
                    nc.vector.tensor_scalar(t1[:], t1[:], tgam[:], tbet[:], AL.mult, AL.add)
                    nc.vector.tensor_tensor(gt[b][:], t1[:],
                                            sgp[:, (2 * ip + b) * 1024:(2 * ip + b + 1) * 1024], AL.mult)
                    yield

                # output projection: yT[o, i] partials
                for oh in range(2):
                    yp = dp.tile([128, 1024], F, tag="dots")
                    for b in range(2):
                        for nt in range(2):
                            nc.tensor.matmul(yp[:, nt * 512:(nt + 1) * 512],
                                             twout[:, b, oh * 128:(oh + 1) * 128],
                                             gt[b][:, nt * 512:(nt + 1) * 512],
                                             start=(b == 0), stop=(b == 1))
                    ys = epi.tile([128, 1024], F, tag="ys")
                    nc.vector.tensor_copy(ys[:], yp[:])
                    nc.sync.dma_start(out=yT[oh * 128:(oh + 1) * 128, i0:i0 + 1024], in_=ys[:])
                    yield

            attention(0)
            # gates directly bank-packed: sgp [32, 4096], block (ip, b) at col
            # 1024*(2*ip+b).  sig = exp(-ln(exp(-(g+bg)) + 1))
            sgp = pers.tile([32, 4096], F, tag="sgp")
            for ip in range(2):
                for b in range(2):
                    ps = dp.tile([128, 1024], F, tag="dots")
                    for nt in range(2):
                        for f in range(2):
                            nc.tensor.matmul(
                                ps[0:32, nt * 512:(nt + 1) * 512],
                                twg[:, f, 32 * b:32 * b + 32],
                                xt[:, f, ip * 1024 + nt * 512:ip * 1024 + (nt + 1) * 512],
                                start=(f == 0), stop=(f == 1))
                    c0 = (2 * ip + b) * 1024
                    nc.scalar.activation(sgp[:, c0:c0 + 1024], ps[0:32, :], AF.Exp,
                                         scale=-1.0, bias=tnbg[:, b:b + 1])
            nc.scalar.activation(sgp[:], sgp[:], AF.Ln, bias=1.0)
            nc.scalar.activation(sgp[:], sgp[:], AF.Exp, scale=-1.0)

            gen0 = epilogue(0)
            attention(1, interleave=gen0)
            for _ in gen0:
                pass
            for _ in epilogue(1):
                pass

    nc.finalize()
    return nc


def _prep_core_inputs(inputs, bi, hg, lam):
    scale = DH ** -0.5
    x = np.asarray(inputs["x"], np.float32)
    Wq = np.asarray(inputs["Wq"], np.float32)
    Wkv = np.asarray(inputs["Wkv"], np.float32)
    Wout = np.asarray(inputs["Wout"], np.float32)
    Wg = np.asarray(inputs["Wg"], np.float32)
    bg = np.asarray(inputs["bg"], np.float32)
    g_ = np.asarray(inputs["ln_gamma"], np.float32)
    b_ = np.asarray(inputs["ln_beta"], np.float32)
    li = np.float32(1.0 - LAMBDA_INIT)

    c0 = 64 * hg
    wq_c = Wq[:, c0:c0 + 64] * scale
    wk_c = Wkv[:, c0:c0 + 64]
    wv_c = Wkv[:, 256 + c0:256 + c0 + 64]
    wg_c = Wg[:, c0:c0 + 64]
    wout_c = Wout[c0:c0 + 64, :]

    wqp = np.zeros((256, 128), np.float32)
    wkp = np.zeros((256, 128), np.float32)
    for d in range(4):
        wqp[:, 32 * d:32 * d + 16] = wq_c[:, 16 * d:16 * d + 16]
        wkp[:, 32 * d:32 * d + 16] = wk_c[:, 16 * d:16 * d + 16]

    woutp = np.zeros((32, 2, 256), np.float32)
    woutp[:, 0, :] = wout_c[0:32, :]
    woutp[:, 1, :] = wout_c[32:64, :]

    sel = np.zeros((4, 256), np.float32)
    sel[0, 0:32] = 1.0
    sel[1, 64:96] = lam
    sel[2, 128:160] = 1.0
    sel[3, 192:224] = lam

    fold = np.zeros((128, 32), np.float32)
    for r in range(32):
        fold[r, r] = 1.0
        fold[64 + r, r] = -1.0

    smu = np.zeros((32, 4), np.float32)
    smu[:, 0] = 1.0 / 32.0
    smu[:, 3] = 1.0 / 32.0

    ones32 = np.ones((1, 32), np.float32)
    onescol = np.zeros((128, 32), ml_dtypes.bfloat16)
    onescol[:, 0] = 1.0

    return {
        "xT": np.ascontiguousarray(x[bi].T),
        "wqp": wqp, "wkp": wkp,
        "wv": np.ascontiguousarray(wv_c),
        "wgc": np.ascontiguousarray(wg_c),
        "wout": woutp,
        "sel": sel, "fold": fold, "smu": smu,
        "ones32": ones32, "onescol": onescol,
        "epsc": np.full((1, 1), LN_EPS, np.float32),
        "gam": (g_[0:32] * li).reshape(32, 1).astype(np.float32),
        "bet": (b_[0:32] * li).reshape(32, 1).astype(np.float32),
        "nbg": (-bg[c0:c0 + 64]).reshape(64, 1).astype(np.float32),
    }


def kernel(**inputs) -> np.ndarray:
    lq1 = np.asarray(inputs["lq1"], np.float64)
    lk1 = np.asarray(inputs["lk1"], np.float64)
    lq2 = np.asarray(inputs["lq2"], np.float64)
    lk2 = np.asarray(inputs["lk2"], np.float64)
    lam = float(np.exp(np.sum(lq1 * lk1)) - np.exp(np.sum(lq2 * lk2)) + LAMBDA_INIT)
    bout = np.asarray(inputs["bout"], np.float32)

    if "nc" not in _cached:
        _cached["nc"] = build_kernel()
    nc = _cached["nc"]

    in_maps = []
    for c in range(NC):
        bi, hg = c // 4, c % 4
        in_maps.append(_prep_core_inputs(inputs, bi, hg, lam))

    import os
    trace = bool(int(os.environ.get("BASS_KERNEL_TRACE", "0")))
    res = run_bass_kernel_spmd(nc, in_maps, list(range(NC)), trace=trace)
    _cached["exec_time_ns"] = res.exec_time_ns
    _cached["trace"] = res.instructions_and_trace
    out = np.zeros((B, N, DIM), np.float32)
    for c in range(NC):
        bi = c // 4
        out[bi] += res.results[c]["yT"].T
    out += bout
    return out
